# revision 15
# baseline (speedup 1.0000x reference)
"""CrossAttention kernel for 8 Trainium2 NeuronCores.

Problem (hardcoded shapes): B=4, N=1024, C=1024, E=1024, H=16, D=64.
  kv = x @ Wkv + bkv ; k, v = split(kv) ; q = query @ Wq + bq
  keys = [k; q] (2N), values = [v; v]
  out = softmax(q keys^T / sqrt(D)) @ values        -> [B, N, E]

Sharding: 8 cores = 4 batches x 2 head-groups (8 heads each).
Per-core strategy (all matmul contractions run on the partition dim):
  - x^T, query^T shipped host-transposed [C, N]
  - q^T, k^T computed on-chip as [Ecol, N] (head-pair-major partitions)
  - scores computed transposed [keys, queries] so the PV matmul needs no
    on-chip transposes; both query blocks share one 2-bank PSUM tile so
    exp runs 1024 wide; softmax denominator comes from a ones-column
    appended to the V stationary; output returned transposed [Ecol, N]
    and un-transposed on the host.
All matmuls run in float32r (full PE rate at fp32 storage, ~1e-4 rel err).
"""
import numpy as np

B, N, C, E, H = 4, 1024, 1024, 1024, 16
D = E // H            # 64
HPC = 8               # heads per core
EC = HPC * D          # 512 E-columns per core
NCORES = 8
CT = C // 128         # 8 contraction tiles
ST = N // 128         # 8 seq tiles
KT = 2 * N // 128     # 16 key tiles (k then q-as-keys)
PAIRS = HPC // 2      # 4 head pairs

_compiled = None


def _build():
    import concourse.bass as bass
    import concourse.bacc as bacc
    import concourse.mybir as mybir
    import concourse.tile as tile
    import contextlib

    F32 = mybir.dt.float32
    F32R = mybir.dt.float32r
    EXP = mybir.ActivationFunctionType.Exp

    nc = bacc.Bacc()
    xT_in = nc.declare_dram_parameter("xT", [C, N], F32R, isOutput=False)
    qryT_in = nc.declare_dram_parameter("qryT", [C, N], F32R, isOutput=False)
    wq_in = nc.declare_dram_parameter("wq", [C, EC], F32R, isOutput=False)
    wk_in = nc.declare_dram_parameter("wk", [C, EC], F32R, isOutput=False)
    wv_in = nc.declare_dram_parameter("wv", [C, EC], F32R, isOutput=False)
    bq_in = nc.declare_dram_parameter("bq", [EC], F32R, isOutput=False)
    bk_in = nc.declare_dram_parameter("bk", [EC], F32R, isOutput=False)
    bv_in = nc.declare_dram_parameter("bv", [EC], F32R, isOutput=False)
    ones_in = nc.declare_dram_parameter("ones", [512], F32R, isOutput=False)
    out_o = nc.declare_dram_parameter("out_t", [EC, N], F32, isOutput=True)

    with tile.TileContext(nc) as tc, contextlib.ExitStack() as ctx:
        pers = ctx.enter_context(tc.tile_pool(name="pers", bufs=1))
        epool = ctx.enter_context(tc.tile_pool(name="epool", bufs=4))
        outp = ctx.enter_context(tc.tile_pool(name="outp", bufs=2))

        # ---- persistent SBUF ----
        xTs = pers.tile([128, CT, N], F32R, tag="xTs")
        qryTs = pers.tile([128, CT, N], F32R, tag="qryTs")
        wqs = pers.tile([128, CT, EC], F32R, tag="wqs")
        wks = pers.tile([128, CT, EC], F32R, tag="wks")
        wvs = pers.tile([128, CT, EC], F32R, tag="wvs")
        qTs = pers.tile([128, PAIRS, N], F32R, tag="qTs")
        kTs = pers.tile([128, PAIRS, N], F32R, tag="kTs")
        vvs = pers.tile([128, ST, HPC, D + 1], F32R, tag="vvs")
        bqr = pers.tile([1, EC], F32R, tag="bqr")
        bkr = pers.tile([1, EC], F32R, tag="bkr")
        bvr = pers.tile([1, EC], F32R, tag="bvr")
        onesr = pers.tile([1, 512], F32R, tag="onesr")

        # ---- loads (SWDGE cast-DMA f32 -> f32r), interleaved in consumption
        # order so the projections start as soon as their first tiles land
        for ct in range(CT):
            nc.sync.dma_start(out=wvs[:, ct, :], in_=wv_in[ct * 128:(ct + 1) * 128, :])
            nc.sync.dma_start(out=xTs[:, ct, :], in_=xT_in[ct * 128:(ct + 1) * 128, :])
            if ct == 0:
                nc.sync.dma_start(out=bqr[:], in_=bq_in.ap().rearrange("(o e) -> o e", o=1))
                nc.sync.dma_start(out=bkr[:], in_=bk_in.ap().rearrange("(o e) -> o e", o=1))
                nc.sync.dma_start(out=bvr[:], in_=bv_in.ap().rearrange("(o e) -> o e", o=1))
                nc.sync.dma_start(out=onesr[:], in_=ones_in.ap().rearrange("(o e) -> o e", o=1))
                _o = ones_in.ap()
                ones_bc = bass.AP(tensor=_o.tensor, offset=_o.offset,
                                  ap=[[0, 128], [ST, HPC], [1, ST], [1, 1]])
                nc.sync.dma_start(out=vvs[:, :, :, D:D + 1], in_=ones_bc)
        for ct in range(CT):
            nc.sync.dma_start(out=wks[:, ct, :], in_=wk_in[ct * 128:(ct + 1) * 128, :])
        for ct in range(CT):
            nc.sync.dma_start(out=wqs[:, ct, :], in_=wq_in[ct * 128:(ct + 1) * 128, :])
            nc.sync.dma_start(out=qryTs[:, ct, :], in_=qryT_in[ct * 128:(ct + 1) * 128, :])

        # ================= phase 1: projections =================
        # ct-outer loops with 8 psum accumulators; weights streamed per ct.
        with tc.tile_pool(name="proj_ps", bufs=8, space="PSUM") as proj_ps:
            # V: v[st] = (x @ Wv + bv), natural [seq, Ecol]
            vps = []
            for st in range(ST):
                vp = proj_ps.tile([128, EC], F32, tag="proj")
                vps.append(vp)
            for ct in range(CT):
                for st in range(ST):
                    nc.tensor.matmul(vps[st][:], xTs[:, ct, st * 128:(st + 1) * 128],
                                     wvs[:, ct, :], start=(ct == 0), stop=False)
            for st in range(ST):
                nc.tensor.matmul(vps[st][:], onesr[:, 0:128], bvr[:], start=False, stop=True)
                nc.vector.tensor_copy(out=vvs[:, st, :, 0:D],
                                      in_=vps[st][:].rearrange("p (h d) -> p h d", h=HPC))
            # q^T / k^T: [Ecol(128), N] per head pair, all pairs at once
            for wsb, srcs, brow, dst in ((wks, xTs, bkr, kTs), (wqs, qryTs, bqr, qTs)):
                pps = []
                for i in range(2 * PAIRS):
                    pp = proj_ps.tile([128, 512], F32, tag="proj")
                    pps.append(pp)
                for ct in range(CT):
                    for pair in range(PAIRS):
                        for qb in range(2):
                            nc.tensor.matmul(pps[2 * pair + qb][:],
                                             wsb[:, ct, pair * 128:(pair + 1) * 128],
                                             srcs[:, ct, qb * 512:(qb + 1) * 512],
                                             start=(ct == 0), stop=False)
                for pair in range(PAIRS):
                    pcols = slice(pair * 128, (pair + 1) * 128)
                    for qb in range(2):
                        pp = pps[2 * pair + qb]
                        nc.tensor.matmul(pp[:], brow[:, pcols], onesr[:],
                                         start=False, stop=True)
                        nc.vector.tensor_copy(out=dst[:, pair, qb * 512:(qb + 1) * 512],
                                              in_=pp[:])

        # ================= phase 2: attention =================
        with tc.tile_pool(name="sc_ps", bufs=2, space="PSUM") as sc_ps, \
             tc.tile_pool(name="pv_ps", bufs=4, space="PSUM") as pv_ps:
            for pair in range(PAIRS):
                pcols = slice(pair * 128, (pair + 1) * 128)
                hA, hB = 2 * pair, 2 * pair + 1
                # 4 accumulators: (head A/B) x (query block 0/1)
                pv = [[None, None], [None, None]]
                for hi in range(2):
                    for qb in range(2):
                        pvt = pv_ps.tile([D + 1, 512], F32, tag="pv")
                        pv[hi][qb] = pvt
                # software-pipelined: scores/exp for kt run ahead of PV for kt-1
                # so the ACT engine (the bottleneck) never waits on the PE.
                prev_e2 = None
                for kt in range(KT):
                    src = kTs if kt < ST else qTs
                    ksl = slice((kt % ST) * 128, (kt % ST + 1) * 128)
                    e2 = []
                    for hi, rows in ((0, slice(0, 64)), (1, slice(64, 128))):
                        s2 = sc_ps.tile([128, 1024], F32, tag="sc")
                        nc.tensor.matmul(s2[:, 0:512], src[rows, pair, ksl],
                                         qTs[rows, pair, 0:512])
                        nc.tensor.matmul(s2[:, 512:1024], src[rows, pair, ksl],
                                         qTs[rows, pair, 512:1024])
                        e = epool.tile([128, 1024], F32R, tag="e")
                        nc.scalar.activation(out=e[:], in_=s2[:], func=EXP, scale=0.125)
                        e2.append(e)
                    if prev_e2 is not None:
                        for hi, h in ((0, hA), (1, hB)):
                            for qb in range(2):
                                nc.tensor.matmul(pv[hi][qb][:], vvs[:, (kt - 1) % ST, h, :],
                                                 prev_e2[hi][:, qb * 512:(qb + 1) * 512],
                                                 start=(kt == 1), stop=False)
                    prev_e2 = e2
                for hi, h in ((0, hA), (1, hB)):
                    for qb in range(2):
                        nc.tensor.matmul(pv[hi][qb][:], vvs[:, (KT - 1) % ST, h, :],
                                         prev_e2[hi][:, qb * 512:(qb + 1) * 512],
                                         start=False, stop=True)
                # normalize: out = pv[0:64] / pv[64]; assemble per-qb [128, 512]
                for qb in range(2):
                    osb = outp.tile([128, 512], F32, tag="osb")
                    for hi in range(2):
                        p = pv[hi][qb]
                        rc = outp.tile([1, 512], F32, tag="rc")
                        nc.vector.reciprocal(out=rc[:], in_=p[D:D + 1, :])
                        bc = outp.tile([64, 512], F32, tag="bc")
                        nc.gpsimd.partition_broadcast(bc[:], rc[:], channels=64)
                        nc.vector.tensor_mul(out=osb[hi * 64:(hi + 1) * 64, :],
                                             in0=p[0:D, :], in1=bc[:])
                    nc.sync.dma_start(out=out_o[pcols, qb * 512:(qb + 1) * 512], in_=osb[:])

    nc.finalize()
    return nc


def _get_compiled():
    global _compiled
    if _compiled is None:
        _compiled = _build()
    return _compiled


def kernel(x, query, Wkv, bkv, Wq, bq):
    from concourse.bass_utils import run_bass_kernel_spmd

    x = np.asarray(x, dtype=np.float32)
    query = np.asarray(query, dtype=np.float32)
    Wkv = np.asarray(Wkv, dtype=np.float32)
    bkv = np.asarray(bkv, dtype=np.float32)
    Wq = np.asarray(Wq, dtype=np.float32)
    bq = np.asarray(bq, dtype=np.float32)

    ones = np.ones((512,), np.float32)
    in_maps = []
    for core in range(NCORES):
        b, hg = core // 2, core % 2
        ecs = slice(hg * EC, (hg + 1) * EC)
        in_maps.append({
            "xT": np.ascontiguousarray(x[b].T),
            "qryT": np.ascontiguousarray(query[b].T),
            "wq": np.ascontiguousarray(Wq[:, ecs]),
            "wk": np.ascontiguousarray(Wkv[:, hg * EC:(hg + 1) * EC]),
            "wv": np.ascontiguousarray(Wkv[:, E + hg * EC:E + (hg + 1) * EC]),
            "bq": np.ascontiguousarray(bq[ecs]),
            "bk": np.ascontiguousarray(bkv[hg * EC:(hg + 1) * EC]),
            "bv": np.ascontiguousarray(bkv[E + hg * EC:E + (hg + 1) * EC]),
            "ones": ones,
        })

    nc = _get_compiled()
    res = None
    last_err = None
    for attempt in range(3):
        try:
            res = run_bass_kernel_spmd(nc, in_maps, list(range(NCORES)))
            break
        except Exception as ex:  # transient NRT_EXEC_UNIT_UNRECOVERABLE etc.
            last_err = ex
    if res is None:
        raise last_err

    out = np.empty((B, N, E), np.float32)
    for core in range(NCORES):
        b, hg = core // 2, core % 2
        out[b, :, hg * EC:(hg + 1) * EC] = res.results[core]["out_t"].T
    return out
